# revision 22
# baseline (speedup 1.0000x reference)
"""Multi-head causal attention with RoPE on 8 Trainium2 NeuronCores.

Problem: B=2, S=2048, D=1024, H=16 heads (dk=64), fp32 in/out, causal mask,
RoPE on Q/K, y = softmax(QK^T/sqrt(dk)) V projected by Wo.

Sharding: head-parallel. Core c owns 2 heads (columns c*128:(c+1)*128 of the
QKV projection output). All matmul operands are bf16 (fp32 PSUM accumulate);
the 2e-2 rel-err budget has ~30x headroom over bf16 noise.

Per core:
  1. Q^T,K^T,V^T [FW, T] for its heads from the full x (K-dim 1024 matmuls,
     512-row moving operands), RoPE applied in the [head_dim, token] layout,
  2. V^T flipped to [token, head_dim] via the DMA xbar transpose (free),
     with ones columns appended so the PV matmul also emits the softmax
     denominator,
  3. causal attention with transposed scores ST[k,q]: exp on ScalarE straight
     out of PSUM (both heads in one ACT for full tiles), diagonal k-tiles
     compute only the valid q-range and get a static 128x128 triangle mask,
  4. denominator handled via reciprocal_approx_fast + a [2,128] selector
     matmul that broadcasts both heads' 1/l across partitions at once,
  5. two 512KB AllToAlls (one per batch) flip head-sharded -> token-sharded;
     the batch-0 collective and its quarter of the output projection hide
     under batch-1 compute. Dummy matmuls keep the PE HAM clock warm across
     the tail collective.
Core c owns tokens [c*256,(c+1)*256) of each batch; host reassembles.
"""

import sys

for p in ("/opt/trn_rl_repo", "/root/.axon_site/_ro/trn_rl_repo"):
    if p not in sys.path:
        sys.path.insert(0, p)

import math

import numpy as np
import ml_dtypes

import concourse.bass as bass
import concourse.tile as tile
from concourse import mybir
from concourse.bass_utils import run_bass_kernel_spmd

N_CORES = 8
B, S, D, H = 2, 2048, 1024, 16
DK = D // H          # 64
HPC = H // N_CORES   # heads per core = 2
FW = HPC * DK        # head-group width per core = 128
T = B * S            # 4096 flattened tokens
TCH = 512            # token chunk for projections
NCH = T // TCH       # 8 chunks
KT = 128             # k tile
QC = 512             # q chunk in attention
TPB = S // N_CORES   # 256 output tokens per core per batch
TSL = B * TPB        # 512 output rows per core

F32 = mybir.dt.float32
BF16 = mybir.dt.bfloat16
BF = np.dtype(ml_dtypes.bfloat16)


def _spill_waits(nc, max_other=1):
    """walrus in this container allows 1 sync-wait per instruction; move
    excess waits onto preceding single-wait NoOps on the same engine."""
    n_new = 0
    for bb in nc.m.functions[0].blocks:
        newlist = []
        changed = False
        for inst in bb.instructions:
            si = inst.sync_info
            if si is not None and si.on_wait and len(si.on_wait) > max_other:
                waits = list(si.on_wait)
                overflow, keep = waits[:-max_other], waits[-max_other:]
                while overflow:
                    chunk, overflow = overflow[:1], overflow[1:]
                    nop = mybir.InstNoOp(
                        name=f"waitspill{n_new}-{inst.name}", ins=[], outs=[]
                    )
                    nop.engine = inst.engine
                    nop.debug = inst.debug
                    nop.sync_info = mybir.SyncInfo(on_wait=chunk, on_update=[])
                    newlist.append(nop)
                    n_new += 1
                si.on_wait = keep
                inst.sync_info = si
                changed = True
            newlist.append(inst)
        if changed:
            bb.instructions = newlist
    return n_new


def build_kernel():
    nc = bass.Bass("TRN2", num_devices=N_CORES)

    x5 = nc.dram_tensor("x5", [NCH, 128, 8, TCH], BF16, kind="ExternalInput")
    wq = nc.dram_tensor("wq", [128, 8, FW], BF16, kind="ExternalInput")  # pre-scaled
    wk = nc.dram_tensor("wk", [128, 8, FW], BF16, kind="ExternalInput")
    wv = nc.dram_tensor("wv", [128, 8, FW], BF16, kind="ExternalInput")
    woT = nc.dram_tensor("woT", [D, D], BF16, kind="ExternalInput")
    ctab = nc.dram_tensor("ctab", [FW, S], BF16, kind="ExternalInput")
    stab = nc.dram_tensor("stab", [FW, S], BF16, kind="ExternalInput")
    tri = nc.dram_tensor("tri", [KT, KT], BF16, kind="ExternalInput")
    ident = nc.dram_tensor("ident", [128, 128], BF16, kind="ExternalInput")
    y = nc.dram_tensor("y", [TSL, D], F32, kind="ExternalOutput")

    with tile.TileContext(nc) as tc:
        with (
            tc.tile_pool(name="const", bufs=1) as const,
            tc.tile_pool(name="xch", bufs=4) as xch,
            tc.tile_pool(name="qk", bufs=1) as qkpool,
            tc.tile_pool(name="tmp", bufs=3) as tmp,
            tc.tile_pool(name="pts", bufs=3) as pts,
            tc.tile_pool(name="lpool", bufs=2) as lpool,
            tc.tile_pool(name="wo", bufs=8) as wopool,
            tc.tile_pool(name="yout", bufs=2) as ypool,
            tc.tile_pool(name="mm", bufs=2, space="PSUM") as mmps,
            tc.tile_pool(name="st", bufs=2, space="PSUM") as stps,
            tc.tile_pool(name="pv", bufs=2, space="PSUM") as pvps,
            tc.tile_pool(name="dram", bufs=1, space="DRAM") as dram,
        ):
            warm_in = dram.tile([8, 16], F32)
            warm_out = dram.tile([8, 16], F32)

            # ---- constants; wq first so chunk 0 can start immediately ----
            wq_sb = const.tile([128, 8, FW], BF16)
            wk_sb = const.tile([128, 8, FW], BF16)
            wv_sb = const.tile([128, 8, FW], BF16)
            nc.sync.dma_start(out=wq_sb, in_=wq[:, :, :])
            nc.scalar.dma_start(out=wk_sb, in_=wk[:, :, :])
            nc.scalar.dma_start(out=wv_sb, in_=wv[:, :, :])
            c_sb = const.tile([FW, S], BF16)
            s_sb = const.tile([FW, S], BF16)
            nc.scalar.dma_start(out=c_sb, in_=ctab[:, :])
            nc.scalar.dma_start(out=s_sb, in_=stab[:, :])
            tri_sb = const.tile([KT, KT], BF16)
            nc.scalar.dma_start(out=tri_sb, in_=tri[:, :])
            id_sb = const.tile([128, 128], BF16)
            nc.scalar.dma_start(out=id_sb, in_=ident[:, :])
            ones_f = const.tile([1, DK], F32)
            nc.vector.memset(ones_f, 1.0)
            ones64 = const.tile([1, DK], BF16)
            nc.vector.tensor_copy(out=ones64, in_=ones_f)

            qT = qkpool.tile([FW, T], BF16, tag="qT")
            kTt = qkpool.tile([FW, T], BF16, tag="kT")
            v_sb = qkpool.tile([128, T // 128, 2 * DK + 2], BF16, tag="v")
            outT = qkpool.tile([FW, T], BF16, tag="outT")
            # bake the ones columns for the softmax denominator
            vones = const.tile([128, T // 128], F32)
            nc.vector.memset(vones, 1.0)
            nc.vector.tensor_copy(out=v_sb[:, :, DK], in_=vones)
            nc.vector.tensor_copy(out=v_sb[:, :, 2 * DK + 1], in_=vones)

            # ---- QKV projections + RoPE (per x chunk) ----
            xtiles = {}

            def load_chunk(ci):
                xc = xch.tile([128, 8, TCH], BF16, tag="x", name="xc")
                nc.sync.dma_start(out=xc[:, 0:4, :], in_=x5[ci, :, 0:4, :])
                nc.scalar.dma_start(out=xc[:, 4:8, :], in_=x5[ci, :, 4:8, :])
                xtiles[ci] = xc

            def do_qkv_chunk(ci):
                t0 = ci * TCH
                sc = (ci % (S // TCH)) * TCH  # position within batch for rope
                xc = xtiles.pop(ci)


                # Q accumulates in "mm", K in "st" so the two groups never
                # stall on the same PSUM slot rotation.
                for which, w_sb, dst in (("q", wq_sb, qT), ("k", wk_sb, kTt)):
                    if which == "q":
                        ps = mmps.tile([FW, TCH], F32, tag="mm", name="qps")
                    else:
                        ks = stps.tile([128, 2, TCH], F32, tag="st", name="kps")
                        ps = ks[:, 0, :]
                    for dt in range(8):
                        nc.tensor.matmul(
                            ps,
                            w_sb[:, dt, :],
                            xc[:, dt, :],
                            start=(dt == 0),
                            stop=(dt == 7),
                        )
                    raw = tmp.tile([FW, TCH], BF16, tag="raw", name="raw")
                    nc.vector.tensor_copy(out=raw, in_=ps)
                    swp = tmp.tile([FW, TCH], BF16, tag="swp", name="swp")
                    # pair swap across partitions via two strided DMAs
                    seng = nc.sync if which == "q" else nc.scalar
                    seng.dma_start(out=swp[0 : FW - 1 : 2, :], in_=raw[1:FW:2, :])
                    seng.dma_start(out=swp[1:FW:2, :], in_=raw[0 : FW - 1 : 2, :])
                    dslice = dst[:, t0 : t0 + TCH]
                    nc.vector.tensor_mul(dslice, raw, c_sb[:, sc : sc + TCH])
                    t2 = tmp.tile([FW, TCH], BF16, tag="ropetmp", name="t2")
                    nc.gpsimd.tensor_mul(t2, swp, s_sb[:, sc : sc + TCH])
                    nc.vector.tensor_add(dslice, dslice, t2)

                if ci + 2 < NCH:
                    load_chunk(ci + 2)
                # V^T [FW, TCH] like Q/K, then DMA-xbar transpose into v_sb
                vps = mmps.tile([FW, TCH], F32, tag="mm", name="vps")
                for dt in range(8):
                    nc.tensor.matmul(
                        vps,
                        wv_sb[:, dt, :],
                        xc[:, dt, :],
                        start=(dt == 0),
                        stop=(dt == 7),
                    )
                vstage = tmp.tile([FW, TCH], BF16, tag="vstage", name="vstage")
                nc.vector.tensor_copy(out=vstage, in_=vps)
                vtr = tmp.tile([128, TCH // 128, 128], BF16, tag="vtr", name="vtr")
                c0 = t0 // 128
                for i in range(TCH // 128):
                    # xbar source must start at partition 0 and the output
                    # must be contiguous, so transpose the full 128-block
                    # then split around the ones columns with a strided copy
                    teng = nc.sync if i % 2 == 0 else nc.scalar
                    teng.dma_start_transpose(
                        out=vtr[:, i, :], in_=vstage[:, i * 128 : (i + 1) * 128]
                    )
                    vview = v_sb[:, c0 + i, :].rearrange(
                        "p (g j) -> p g j", j=DK + 1
                    )[:, :, 0:DK]
                    nc.scalar.copy(
                        out=vview,
                        in_=vtr[:, i, :].rearrange("p (g j) -> p g j", j=DK),
                    )


            # ---- causal attention for one batch ----
            # Transposed-scores flash style, software-pipelined one k-tile
            # ahead. Diagonal k-tiles only compute the valid q-range.
            def emit_st(b, qc, kt):
                trow = b * S + qc * QC
                kcol = b * S + kt * KT
                o = max(0, (kt - 4 * qc) * KT)
                w = QC - o
                st = stps.tile([128, 2, QC], F32, tag="st", name="st")
                for h2 in range(HPC):
                    fb = h2 * DK
                    nc.tensor.matmul(
                        st[:, h2, 0:w],
                        kTt[fb : fb + DK, kcol : kcol + KT],
                        qT[fb : fb + DK, trow + o : trow + QC],
                        start=True,
                        stop=True,
                    )
                pt = pts.tile([128, 2, QC], BF16, tag="pt", name="pt")
                nc.scalar.activation(
                    out=pt[:, :, 0:w], in_=st[:, :, 0:w],
                    func=mybir.ActivationFunctionType.Exp,
                )
                if kt >= 4 * qc:  # diagonal: mask the triangular 128-col block
                    for h2 in range(HPC):
                        nc.gpsimd.tensor_mul(
                            pt[:, h2, 0:KT], pt[:, h2, 0:KT], tri_sb
                        )
                return pt, o, w

            def emit_pv(b, qc, kt, pv2, prev):
                pt, o, w = prev
                nkt = 4 * (qc + 1)
                for h2 in range(HPC):
                    vcol = h2 * (DK + 1)
                    nc.tensor.matmul(
                        pv2[h2][:, o : o + w],
                        v_sb[:, b * (S // 128) + kt, vcol : vcol + DK + 1],
                        pt[:, h2, 0:w],
                        start=(kt == 0),
                        stop=(kt == nkt - 1),
                        skip_group_check=True,
                    )

            def do_attn_qc(b, qc):
                    trow = b * S + qc * QC
                    pv2 = [
                        pvps.tile([DK + 1, QC], F32, tag="pv", name=f"pv{h2}")
                        for h2 in range(HPC)
                    ]
                    nkt = 4 * (qc + 1)
                    prev = emit_st(b, qc, 0)
                    for kt in range(1, nkt):
                        cur = emit_st(b, qc, kt)
                        emit_pv(b, qc, kt - 1, pv2, prev)
                        prev = cur
                    emit_pv(b, qc, nkt - 1, pv2, prev)
                    # normalization: evacuate pv to SBUF bf16 right away
                    # (frees the PSUM bank). The softmax denominators l sit as
                    # [1, 512] rows, where any elementwise op runs on a single
                    # lane; PE-transpose them into [128, 2x4] so one cheap
                    # reciprocal covers the whole q-chunk, transpose back, and
                    # broadcast across partitions with a K=1 ones matmul.
                    pvcs = []
                    lT = mmps.tile([128, HPC, 4, 2], BF16, tag="mm", name="lT")
                    for h2 in range(HPC):
                        pvc = lpool.tile([DK + 1, QC], BF16, tag=f"pvc{h2}", name="pvc")
                        nc.vector.tensor_copy(out=pvc, in_=pv2[h2])
                        pvcs.append(pvc)
                        for blk in range(4):
                            nc.tensor.transpose(
                                lT[:, h2, blk, 0:1],
                                pvc[DK : DK + 1, blk * 128 : (blk + 1) * 128],
                                id_sb[DK : DK + 1, DK : DK + 1],
                            )
                    linvT = lpool.tile([128, HPC, 4], BF16, tag="linvT", name="linvT")
                    with nc.allow_low_precision(reason="bf16 1/l"):
                        nc.vector.reciprocal(out=linvT, in_=lT[:, :, :, 0])
                    for h2 in range(HPC):
                        fb = h2 * DK
                        linvR = mmps.tile([1, 4, 128], BF16, tag="mm", name="linvR")
                        for blk in range(4):
                            nc.tensor.transpose(
                                linvR[:, blk, :],
                                linvT[:, h2, blk : blk + 1],
                                id_sb,
                            )
                        linv_sb = lpool.tile([1, QC], BF16, tag="linv", name="linv_sb")
                        nc.vector.tensor_copy(
                            out=linv_sb, in_=linvR.rearrange("p a f -> p (a f)")
                        )
                        lb = mmps.tile([DK, QC], F32, tag="mm", name="lb")
                        nc.tensor.matmul(lb, ones64, linv_sb, start=True, stop=True)
                        lb_sb = lpool.tile([DK, QC], BF16, tag="lb", name="lb_sb")
                        nc.vector.tensor_copy(out=lb_sb, in_=lb)
                        nc.gpsimd.tensor_mul(
                            outT[fb : fb + DK, trow : trow + QC],
                            pvcs[h2][0:DK, :],
                            lb_sb,
                        )

            # ---- output projection of one batch-half of this core's tokens ----
            def do_proj(half, orecv):
                for tt in range(TPB // 128):
                    ysb = ypool.tile([128, D], F32, tag="y")
                    for ec in range(D // 512):
                        yps = mmps.tile([128, 512], F32, tag="mm")
                        for p in range(N_CORES):
                            nc.tensor.matmul(
                                yps,
                                orecv[:, p, tt * 128 : (tt + 1) * 128],
                                wo_sb[p][:, ec * 512 : (ec + 1) * 512],
                                start=(p == 0),
                                stop=(p == N_CORES - 1),
                            )
                        nc.vector.tensor_copy(
                            out=ysb[:, ec * 512 : (ec + 1) * 512], in_=yps
                        )
                    r0 = half * TPB + tt * 128
                    nc.sync.dma_start(out=y[r0 : r0 + 128, :], in_=ysb)

            def emit_a2a(b, cc_in, cc_out, orecv):
                for p in range(N_CORES):
                    nc.gpsimd.dma_start(
                        out=cc_in[p, :, :],
                        in_=outT[:, b * S + p * TPB : b * S + (p + 1) * TPB],
                    )
                nc.gpsimd.collective_compute(
                    "AllToAll",
                    mybir.AluOpType.bypass,
                    replica_groups=[list(range(N_CORES))],
                    ins=[cc_in[:].opt()],
                    outs=[cc_out[:].opt()],
                )
                for p in range(N_CORES):
                    nc.gpsimd.dma_start(out=orecv[:, p, :], in_=cc_out[p, :, :])

            cc_in0 = dram.tile([N_CORES, FW, TPB], BF16)
            cc_out0 = dram.tile([N_CORES, FW, TPB], BF16)
            cc_in1 = dram.tile([N_CORES, FW, TPB], BF16)
            cc_out1 = dram.tile([N_CORES, FW, TPB], BF16)
            orecv0 = qkpool.tile([128, N_CORES, TPB], BF16, tag="or0")
            orecv1 = qkpool.tile([128, N_CORES, TPB], BF16, tag="or1")

            # Batch-0 QKV (chunks 0-3) runs 10-30us with all its swap and
            # V-transpose DMAs clear of the one-time CC-setup ring freeze
            # (~45-70us); the DMA-free attention(b0) phase then spans the
            # freeze. Batch-1 x is prefetched; its chunks run post-freeze.
            load_chunk(0)
            load_chunk(1)
            for ci in range(NCH // 2):
                do_qkv_chunk(ci)
            wtile = const.tile([1, 128], F32)
            nc.vector.memset(wtile, 0.0)
            nc.gpsimd.dma_start(out=warm_in[:, :], in_=wtile[:1, :128].rearrange("p (a f) -> (p a) f", a=8))
            nc.gpsimd.collective_compute(
                "AllToAll",
                mybir.AluOpType.bypass,
                replica_groups=[list(range(N_CORES))],
                ins=[warm_in[:].opt()],
                outs=[warm_out[:].opt()],
            )
            for qc in range(4):
                do_attn_qc(0, qc)
            emit_a2a(0, cc_in0, cc_out0, orecv0)
            wo_sb = []
            for p in range(N_CORES):
                wt = wopool.tile([128, D], BF16, tag="wo")
                nc.scalar.dma_start(out=wt, in_=woT[p * 128 : (p + 1) * 128, :])
                wo_sb.append(wt)
            for ci in range(NCH // 2, NCH):
                do_qkv_chunk(ci)
            do_proj(0, orecv0)
            for qc in range(4):
                do_attn_qc(1, qc)
            emit_a2a(1, cc_in1, cc_out1, orecv1)
            # dummy matmuls keep the HAM clock warm across the tail collective
            for i in range(6):
                warmps = mmps.tile([FW, TCH], F32, tag="mm", name="warm")
                nc.tensor.matmul(
                    warmps, wq_sb[:, i, :], c_sb[:, 0:TCH],
                    start=True, stop=True,
                )
            do_proj(1, orecv1)

    _spill_waits(nc)
    return nc


_NC_CACHE = None


def _get_nc():
    global _NC_CACHE
    if _NC_CACHE is None:
        _NC_CACHE = build_kernel()
    return _NC_CACHE


def _host_prep(x, Wq, Wk, Wv, Wo, token_positions):
    xT = np.ascontiguousarray(x.reshape(T, D).T)  # [D, T]
    # x5[ci, p, dt, t] = xT[dt*128 + p, ci*TCH + t]
    x5 = np.ascontiguousarray(
        xT.reshape(8, 128, NCH, TCH).transpose(2, 1, 0, 3)
    ).astype(BF)
    WqT = Wq.T * np.float32(1.0 / math.sqrt(DK))
    WkT = Wk.T
    WvT = Wv.T
    WoT = np.ascontiguousarray(Wo.T).astype(BF)

    pos = token_positions.astype(np.float64)  # [S]
    i = (np.arange(FW) % DK) // 2  # pair index per row
    inv_freq = 1.0 / (10000.0 ** (2.0 * i / DK))  # [FW]
    ang = inv_freq[:, None] * pos[None, :]  # [FW, S]
    ctab = np.cos(ang).astype(BF)
    sgn = np.where(np.arange(FW) % 2 == 0, -1.0, 1.0)
    stab = (np.sin(ang) * sgn[:, None]).astype(BF)

    tri = (np.arange(KT)[None, :] >= np.arange(KT)[:, None]).astype(BF)
    ident = np.eye(128).astype(BF)
    return x5, WqT, WkT, WvT, WoT, ctab, stab, tri, ident


def _wslice(WT, c):
    # [D, FW] -> [128, 8, FW] with d = dt*128 + p
    ws = WT[:, c * FW : (c + 1) * FW]
    return np.ascontiguousarray(ws.reshape(8, 128, FW).transpose(1, 0, 2)).astype(BF)


def kernel(x, Wq, Wk, Wv, Wo, mask, token_positions, num_heads, **run_kw):
    x = np.asarray(x)
    assert int(num_heads) == H and x.shape == (B, S, D)
    x5, WqT, WkT, WvT, WoT, ctab, stab, tri, ident = _host_prep(
        np.asarray(x, np.float32),
        np.asarray(Wq, np.float32),
        np.asarray(Wk, np.float32),
        np.asarray(Wv, np.float32),
        np.asarray(Wo, np.float32),
        np.asarray(token_positions),
    )
    in_maps = []
    for c in range(N_CORES):
        in_maps.append(
            {
                "x5": x5,
                "wq": _wslice(WqT, c),
                "wk": _wslice(WkT, c),
                "wv": _wslice(WvT, c),
                "woT": WoT,
                "ctab": ctab,
                "stab": stab,
                "tri": tri,
                "ident": ident,
            }
        )
    nc = _get_nc()
    res = run_bass_kernel_spmd(
        nc, in_maps, core_ids=list(range(N_CORES)), **run_kw
    )
    out = np.empty((B, S, D), dtype=np.float32)
    for c in range(N_CORES):
        yc = res.results[c]["y"]
        out[0, c * TPB : (c + 1) * TPB, :] = yc[0:TPB]
        out[1, c * TPB : (c + 1) * TPB, :] = yc[TPB:TSL]
    kernel.last_results = res
    return out


# revision 23
# speedup vs baseline: 1.2994x; 1.2994x over previous
"""Multi-head causal attention with RoPE on 8 Trainium2 NeuronCores.

Problem: B=2, S=2048, D=1024, H=16 heads (dk=64), fp32 in/out, causal mask,
RoPE on Q/K, y = softmax(QK^T/sqrt(dk)) V projected by Wo.

Sharding: head-parallel. Core c owns 2 heads (columns c*128:(c+1)*128 of the
QKV projection output). All matmul operands are bf16 (fp32 PSUM accumulate);
the 2e-2 rel-err budget has ~30x headroom over bf16 noise.

Per core:
  1. Q^T,K^T,V^T [FW, T] for its heads from the full x (K-dim 1024 matmuls,
     512-row moving operands), RoPE applied in the [head_dim, token] layout,
  2. V^T flipped to [token, head_dim] via the DMA xbar transpose (free),
     with ones columns appended so the PV matmul also emits the softmax
     denominator,
  3. causal attention with transposed scores ST[k,q]: exp on ScalarE straight
     out of PSUM (both heads in one ACT for full tiles), diagonal k-tiles
     compute only the valid q-range and get a static 128x128 triangle mask,
  4. denominator handled via reciprocal_approx_fast + a [2,128] selector
     matmul that broadcasts both heads' 1/l across partitions at once,
  5. two 512KB AllToAlls (one per batch) flip head-sharded -> token-sharded;
     the batch-0 collective and its quarter of the output projection hide
     under batch-1 compute. Dummy matmuls keep the PE HAM clock warm across
     the tail collective.
Core c owns tokens [c*256,(c+1)*256) of each batch; host reassembles.
"""

import sys

for p in ("/opt/trn_rl_repo", "/root/.axon_site/_ro/trn_rl_repo"):
    if p not in sys.path:
        sys.path.insert(0, p)

import math

import numpy as np
import ml_dtypes

import concourse.bass as bass
import concourse.tile as tile
from concourse import mybir
from concourse.bass_utils import run_bass_kernel_spmd

N_CORES = 8
B, S, D, H = 2, 2048, 1024, 16
DK = D // H          # 64
HPC = H // N_CORES   # heads per core = 2
FW = HPC * DK        # head-group width per core = 128
T = B * S            # 4096 flattened tokens
TCH = 512            # token chunk for projections
NCH = T // TCH       # 8 chunks
KT = 128             # k tile
QC = 512             # q chunk in attention
TPB = S // N_CORES   # 256 output tokens per core per batch
TSL = B * TPB        # 512 output rows per core

F32 = mybir.dt.float32
BF16 = mybir.dt.bfloat16
BF = np.dtype(ml_dtypes.bfloat16)


def _spill_waits(nc, max_other=1):
    """walrus in this container allows 1 sync-wait per instruction; move
    excess waits onto preceding single-wait NoOps on the same engine."""
    n_new = 0
    for bb in nc.m.functions[0].blocks:
        newlist = []
        changed = False
        for inst in bb.instructions:
            si = inst.sync_info
            if si is not None and si.on_wait and len(si.on_wait) > max_other:
                waits = list(si.on_wait)
                overflow, keep = waits[:-max_other], waits[-max_other:]
                while overflow:
                    chunk, overflow = overflow[:1], overflow[1:]
                    nop = mybir.InstNoOp(
                        name=f"waitspill{n_new}-{inst.name}", ins=[], outs=[]
                    )
                    nop.engine = inst.engine
                    nop.debug = inst.debug
                    nop.sync_info = mybir.SyncInfo(on_wait=chunk, on_update=[])
                    newlist.append(nop)
                    n_new += 1
                si.on_wait = keep
                inst.sync_info = si
                changed = True
            newlist.append(inst)
        if changed:
            bb.instructions = newlist
    return n_new


def build_kernel():
    nc = bass.Bass("TRN2", num_devices=N_CORES)

    x5 = nc.dram_tensor("x5", [NCH, 128, 8, TCH], BF16, kind="ExternalInput")
    wq = nc.dram_tensor("wq", [128, 8, FW], BF16, kind="ExternalInput")  # pre-scaled
    wk = nc.dram_tensor("wk", [128, 8, FW], BF16, kind="ExternalInput")
    wv = nc.dram_tensor("wv", [128, 8, FW], BF16, kind="ExternalInput")
    woT = nc.dram_tensor("woT", [D, D], BF16, kind="ExternalInput")
    ctab = nc.dram_tensor("ctab", [FW, S], BF16, kind="ExternalInput")
    stab = nc.dram_tensor("stab", [FW, S], BF16, kind="ExternalInput")
    tri = nc.dram_tensor("tri", [KT, KT], BF16, kind="ExternalInput")
    ident = nc.dram_tensor("ident", [128, 128], BF16, kind="ExternalInput")
    perm = nc.dram_tensor("perm", [128, 128], BF16, kind="ExternalInput")
    y = nc.dram_tensor("y", [TSL, D], F32, kind="ExternalOutput")

    with tile.TileContext(nc) as tc:
        with (
            tc.tile_pool(name="const", bufs=1) as const,
            tc.tile_pool(name="xch", bufs=4) as xch,
            tc.tile_pool(name="qk", bufs=1) as qkpool,
            tc.tile_pool(name="tmp", bufs=3) as tmp,
            tc.tile_pool(name="pts", bufs=3) as pts,
            tc.tile_pool(name="lpool", bufs=2) as lpool,
            tc.tile_pool(name="wo", bufs=8) as wopool,
            tc.tile_pool(name="yout", bufs=2) as ypool,
            tc.tile_pool(name="mm", bufs=2, space="PSUM") as mmps,
            tc.tile_pool(name="st", bufs=2, space="PSUM") as stps,
            tc.tile_pool(name="pv", bufs=2, space="PSUM") as pvps,
            tc.tile_pool(name="dram", bufs=1, space="DRAM") as dram,
        ):
            warm_in = dram.tile([8, 16], F32)
            warm_out = dram.tile([8, 16], F32)

            # ---- constants; wq first so chunk 0 can start immediately ----
            wq_sb = const.tile([128, 8, FW], BF16)
            wk_sb = const.tile([128, 8, FW], BF16)
            wv_sb = const.tile([128, 8, FW], BF16)
            nc.sync.dma_start(out=wq_sb, in_=wq[:, :, :])
            nc.scalar.dma_start(out=wk_sb, in_=wk[:, :, :])
            nc.scalar.dma_start(out=wv_sb, in_=wv[:, :, :])
            c_sb = const.tile([FW, S], BF16)
            s_sb = const.tile([FW, S], BF16)
            nc.scalar.dma_start(out=c_sb, in_=ctab[:, :])
            nc.scalar.dma_start(out=s_sb, in_=stab[:, :])
            tri_sb = const.tile([KT, KT], BF16)
            nc.scalar.dma_start(out=tri_sb, in_=tri[:, :])
            id_sb = const.tile([128, 128], BF16)
            nc.scalar.dma_start(out=id_sb, in_=ident[:, :])
            pm_sb = const.tile([128, 128], BF16)
            nc.scalar.dma_start(out=pm_sb, in_=perm[:, :])
            ones_f = const.tile([1, DK], F32)
            nc.vector.memset(ones_f, 1.0)
            ones64 = const.tile([1, DK], BF16)
            nc.vector.tensor_copy(out=ones64, in_=ones_f)

            qT = qkpool.tile([FW, T], BF16, tag="qT")
            kTt = qkpool.tile([FW, T], BF16, tag="kT")
            v_sb = qkpool.tile([128, T // 128, 2 * DK + 2], BF16, tag="v")
            outT = qkpool.tile([FW, T], BF16, tag="outT")
            # bake the ones columns for the softmax denominator
            vones = const.tile([128, T // 128], F32)
            nc.vector.memset(vones, 1.0)
            nc.vector.tensor_copy(out=v_sb[:, :, DK], in_=vones)
            nc.vector.tensor_copy(out=v_sb[:, :, 2 * DK + 1], in_=vones)

            # ---- QKV projections + RoPE (per x chunk) ----
            xtiles = {}

            def load_chunk(ci):
                xc = xch.tile([128, 8, TCH], BF16, tag="x", name="xc")
                nc.sync.dma_start(out=xc[:, 0:4, :], in_=x5[ci, :, 0:4, :])
                nc.scalar.dma_start(out=xc[:, 4:8, :], in_=x5[ci, :, 4:8, :])
                xtiles[ci] = xc

            def do_qkv_chunk(ci):
                t0 = ci * TCH
                sc = (ci % (S // TCH)) * TCH  # position within batch for rope
                xc = xtiles.pop(ci)


                # Q accumulates in "mm", K in "st" so the two groups never
                # stall on the same PSUM slot rotation.
                for which, w_sb, dst in (("q", wq_sb, qT), ("k", wk_sb, kTt)):
                    if which == "q":
                        ps = mmps.tile([FW, TCH], F32, tag="mm", name="qps")
                    else:
                        ks = stps.tile([128, 2, TCH], F32, tag="st", name="kps")
                        ps = ks[:, 0, :]
                    for dt in range(8):
                        nc.tensor.matmul(
                            ps,
                            w_sb[:, dt, :],
                            xc[:, dt, :],
                            start=(dt == 0),
                            stop=(dt == 7),
                        )
                    raw = tmp.tile([FW, TCH], BF16, tag="raw", name="raw")
                    nc.vector.tensor_copy(out=raw, in_=ps)
                    # pair swap across partitions via a PE permutation matmul
                    # (no ring DMA: keeps rope immune to the CC ring freeze)
                    swp = mmps.tile([FW, TCH], F32, tag="mm", name="swp")
                    nc.tensor.matmul(swp, pm_sb, raw, start=True, stop=True)
                    dslice = dst[:, t0 : t0 + TCH]
                    nc.vector.tensor_mul(dslice, raw, c_sb[:, sc : sc + TCH])
                    t2 = tmp.tile([FW, TCH], BF16, tag="ropetmp", name="t2")
                    nc.vector.tensor_mul(t2, swp, s_sb[:, sc : sc + TCH])
                    nc.vector.tensor_add(dslice, dslice, t2)

                if ci + 2 < NCH:
                    load_chunk(ci + 2)
                # V^T [FW, TCH] like Q/K, then DMA-xbar transpose into v_sb
                vps = mmps.tile([FW, TCH], F32, tag="mm", name="vps")
                for dt in range(8):
                    nc.tensor.matmul(
                        vps,
                        wv_sb[:, dt, :],
                        xc[:, dt, :],
                        start=(dt == 0),
                        stop=(dt == 7),
                    )
                vstage = tmp.tile([FW, TCH], BF16, tag="vstage", name="vstage")
                nc.scalar.copy(out=vstage, in_=vps)
                vtp = mmps.tile([128, TCH // 128, 128], BF16, tag="mm", name="vtp")
                c0 = t0 // 128
                for i in range(TCH // 128):
                    # PE identity transpose (ring-DMA-free), then split around
                    # the ones columns with a strided ScalarE copy
                    nc.tensor.transpose(
                        vtp[:, i, :], vstage[:, i * 128 : (i + 1) * 128], id_sb
                    )
                    vview = v_sb[:, c0 + i, :].rearrange(
                        "p (g j) -> p g j", j=DK + 1
                    )[:, :, 0:DK]
                    nc.scalar.copy(
                        out=vview,
                        in_=vtp[:, i, :].rearrange("p (g j) -> p g j", j=DK),
                    )


            # ---- causal attention for one batch ----
            # Transposed-scores flash style, software-pipelined one k-tile
            # ahead. Diagonal k-tiles only compute the valid q-range.
            def emit_st(b, qc, kt):
                trow = b * S + qc * QC
                kcol = b * S + kt * KT
                o = max(0, (kt - 4 * qc) * KT)
                w = QC - o
                st = stps.tile([128, 2, QC], F32, tag="st", name="st")
                for h2 in range(HPC):
                    fb = h2 * DK
                    nc.tensor.matmul(
                        st[:, h2, 0:w],
                        kTt[fb : fb + DK, kcol : kcol + KT],
                        qT[fb : fb + DK, trow + o : trow + QC],
                        start=True,
                        stop=True,
                    )
                pt = pts.tile([128, 2, QC], BF16, tag="pt", name="pt")
                nc.scalar.activation(
                    out=pt[:, :, 0:w], in_=st[:, :, 0:w],
                    func=mybir.ActivationFunctionType.Exp,
                )
                if kt >= 4 * qc:  # diagonal: mask the triangular 128-col block
                    for h2 in range(HPC):
                        nc.gpsimd.tensor_mul(
                            pt[:, h2, 0:KT], pt[:, h2, 0:KT], tri_sb
                        )
                return pt, o, w

            def emit_pv(b, qc, kt, pv2, prev):
                pt, o, w = prev
                nkt = 4 * (qc + 1)
                for h2 in range(HPC):
                    vcol = h2 * (DK + 1)
                    nc.tensor.matmul(
                        pv2[h2][:, o : o + w],
                        v_sb[:, b * (S // 128) + kt, vcol : vcol + DK + 1],
                        pt[:, h2, 0:w],
                        start=(kt == 0),
                        stop=(kt == nkt - 1),
                        skip_group_check=True,
                    )

            def do_attn_qc(b, qc):
                    trow = b * S + qc * QC
                    pv2 = [
                        pvps.tile([DK + 1, QC], F32, tag="pv", name=f"pv{h2}")
                        for h2 in range(HPC)
                    ]
                    nkt = 4 * (qc + 1)
                    prev = emit_st(b, qc, 0)
                    for kt in range(1, nkt):
                        cur = emit_st(b, qc, kt)
                        emit_pv(b, qc, kt - 1, pv2, prev)
                        prev = cur
                    emit_pv(b, qc, nkt - 1, pv2, prev)
                    # normalization: evacuate pv to SBUF bf16 right away
                    # (frees the PSUM bank). The softmax denominators l sit as
                    # [1, 512] rows, where any elementwise op runs on a single
                    # lane; PE-transpose them into [128, 2x4] so one cheap
                    # reciprocal covers the whole q-chunk, transpose back, and
                    # broadcast across partitions with a K=1 ones matmul.
                    pvcs = []
                    lT = mmps.tile([128, HPC, 4, 2], BF16, tag="mm", name="lT")
                    for h2 in range(HPC):
                        pvc = lpool.tile([DK + 1, QC], BF16, tag=f"pvc{h2}", name="pvc")
                        nc.vector.tensor_copy(out=pvc, in_=pv2[h2])
                        pvcs.append(pvc)
                        for blk in range(4):
                            nc.tensor.transpose(
                                lT[:, h2, blk, 0:1],
                                pvc[DK : DK + 1, blk * 128 : (blk + 1) * 128],
                                id_sb[DK : DK + 1, DK : DK + 1],
                            )
                    linvT = lpool.tile([128, HPC, 4], BF16, tag="linvT", name="linvT")
                    with nc.allow_low_precision(reason="bf16 1/l"):
                        nc.vector.reciprocal(out=linvT, in_=lT[:, :, :, 0])
                    for h2 in range(HPC):
                        fb = h2 * DK
                        linvR = mmps.tile([1, 4, 128], BF16, tag="mm", name="linvR")
                        for blk in range(4):
                            nc.tensor.transpose(
                                linvR[:, blk, :],
                                linvT[:, h2, blk : blk + 1],
                                id_sb,
                            )
                        linv_sb = lpool.tile([1, QC], BF16, tag="linv", name="linv_sb")
                        nc.vector.tensor_copy(
                            out=linv_sb, in_=linvR.rearrange("p a f -> p (a f)")
                        )
                        lb = mmps.tile([DK, QC], F32, tag="mm", name="lb")
                        nc.tensor.matmul(lb, ones64, linv_sb, start=True, stop=True)
                        lb_sb = lpool.tile([DK, QC], BF16, tag="lb", name="lb_sb")
                        nc.vector.tensor_copy(out=lb_sb, in_=lb)
                        nc.gpsimd.tensor_mul(
                            outT[fb : fb + DK, trow : trow + QC],
                            pvcs[h2][0:DK, :],
                            lb_sb,
                        )

            # ---- output projection of one batch-half of this core's tokens ----
            def do_proj(half, orecv):
                for tt in range(TPB // 128):
                    ysb = ypool.tile([128, D], F32, tag="y")
                    for ec in range(D // 512):
                        yps = mmps.tile([128, 512], F32, tag="mm")
                        for p in range(N_CORES):
                            nc.tensor.matmul(
                                yps,
                                orecv[:, p, tt * 128 : (tt + 1) * 128],
                                wo_sb[p][:, ec * 512 : (ec + 1) * 512],
                                start=(p == 0),
                                stop=(p == N_CORES - 1),
                            )
                        nc.vector.tensor_copy(
                            out=ysb[:, ec * 512 : (ec + 1) * 512], in_=yps
                        )
                    r0 = half * TPB + tt * 128
                    nc.sync.dma_start(out=y[r0 : r0 + 128, :], in_=ysb)

            def emit_a2a(b, cc_in, cc_out, orecv):
                for p in range(N_CORES):
                    nc.gpsimd.dma_start(
                        out=cc_in[p, :, :],
                        in_=outT[:, b * S + p * TPB : b * S + (p + 1) * TPB],
                    )
                nc.gpsimd.collective_compute(
                    "AllToAll",
                    mybir.AluOpType.bypass,
                    replica_groups=[list(range(N_CORES))],
                    ins=[cc_in[:].opt()],
                    outs=[cc_out[:].opt()],
                )
                for p in range(N_CORES):
                    nc.gpsimd.dma_start(out=orecv[:, p, :], in_=cc_out[p, :, :])

            cc_in0 = dram.tile([N_CORES, FW, TPB], BF16)
            cc_out0 = dram.tile([N_CORES, FW, TPB], BF16)
            cc_in1 = dram.tile([N_CORES, FW, TPB], BF16)
            cc_out1 = dram.tile([N_CORES, FW, TPB], BF16)
            orecv0 = qkpool.tile([128, N_CORES, TPB], BF16, tag="or0")
            orecv1 = qkpool.tile([128, N_CORES, TPB], BF16, tag="or1")

            # Batch-0 QKV (chunks 0-3) runs 10-30us with all its swap and
            # V-transpose DMAs clear of the one-time CC-setup ring freeze
            # (~45-70us); the DMA-free attention(b0) phase then spans the
            # freeze. Batch-1 x is prefetched; its chunks run post-freeze.
            load_chunk(0)
            load_chunk(1)
            for ci in range(NCH // 2):
                do_qkv_chunk(ci)
            wtile = const.tile([1, 128], F32)
            nc.vector.memset(wtile, 0.0)
            nc.gpsimd.dma_start(out=warm_in[:, :], in_=wtile[:1, :128].rearrange("p (a f) -> (p a) f", a=8))
            nc.gpsimd.collective_compute(
                "AllToAll",
                mybir.AluOpType.bypass,
                replica_groups=[list(range(N_CORES))],
                ins=[warm_in[:].opt()],
                outs=[warm_out[:].opt()],
            )
            for qc in range(4):
                do_attn_qc(0, qc)
            emit_a2a(0, cc_in0, cc_out0, orecv0)
            wo_sb = []
            for p in range(N_CORES):
                wt = wopool.tile([128, D], BF16, tag="wo")
                nc.scalar.dma_start(out=wt, in_=woT[p * 128 : (p + 1) * 128, :])
                wo_sb.append(wt)
            for ci in range(NCH // 2, NCH):
                do_qkv_chunk(ci)
            do_proj(0, orecv0)
            for qc in range(4):
                do_attn_qc(1, qc)
            emit_a2a(1, cc_in1, cc_out1, orecv1)
            # dummy matmuls keep the HAM clock warm across the tail collective
            for i in range(6):
                warmps = mmps.tile([FW, TCH], F32, tag="mm", name="warm")
                nc.tensor.matmul(
                    warmps, wq_sb[:, i, :], c_sb[:, 0:TCH],
                    start=True, stop=True,
                )
            do_proj(1, orecv1)

    _spill_waits(nc)
    return nc


_NC_CACHE = None


def _get_nc():
    global _NC_CACHE
    if _NC_CACHE is None:
        _NC_CACHE = build_kernel()
    return _NC_CACHE


def _host_prep(x, Wq, Wk, Wv, Wo, token_positions):
    xT = np.ascontiguousarray(x.reshape(T, D).T)  # [D, T]
    # x5[ci, p, dt, t] = xT[dt*128 + p, ci*TCH + t]
    x5 = np.ascontiguousarray(
        xT.reshape(8, 128, NCH, TCH).transpose(2, 1, 0, 3)
    ).astype(BF)
    WqT = Wq.T * np.float32(1.0 / math.sqrt(DK))
    WkT = Wk.T
    WvT = Wv.T
    WoT = np.ascontiguousarray(Wo.T).astype(BF)

    pos = token_positions.astype(np.float64)  # [S]
    i = (np.arange(FW) % DK) // 2  # pair index per row
    inv_freq = 1.0 / (10000.0 ** (2.0 * i / DK))  # [FW]
    ang = inv_freq[:, None] * pos[None, :]  # [FW, S]
    ctab = np.cos(ang).astype(BF)
    sgn = np.where(np.arange(FW) % 2 == 0, -1.0, 1.0)
    stab = (np.sin(ang) * sgn[:, None]).astype(BF)

    tri = (np.arange(KT)[None, :] >= np.arange(KT)[:, None]).astype(BF)
    ident = np.eye(128).astype(BF)
    pr = np.arange(128) ^ 1
    perm = np.zeros((128, 128), np.float32)
    perm[pr, np.arange(128)] = 1.0
    perm = perm.astype(BF)
    return x5, WqT, WkT, WvT, WoT, ctab, stab, tri, ident, perm


def _wslice(WT, c):
    # [D, FW] -> [128, 8, FW] with d = dt*128 + p
    ws = WT[:, c * FW : (c + 1) * FW]
    return np.ascontiguousarray(ws.reshape(8, 128, FW).transpose(1, 0, 2)).astype(BF)


def kernel(x, Wq, Wk, Wv, Wo, mask, token_positions, num_heads, **run_kw):
    x = np.asarray(x)
    assert int(num_heads) == H and x.shape == (B, S, D)
    x5, WqT, WkT, WvT, WoT, ctab, stab, tri, ident, perm = _host_prep(
        np.asarray(x, np.float32),
        np.asarray(Wq, np.float32),
        np.asarray(Wk, np.float32),
        np.asarray(Wv, np.float32),
        np.asarray(Wo, np.float32),
        np.asarray(token_positions),
    )
    in_maps = []
    for c in range(N_CORES):
        in_maps.append(
            {
                "x5": x5,
                "wq": _wslice(WqT, c),
                "wk": _wslice(WkT, c),
                "wv": _wslice(WvT, c),
                "woT": WoT,
                "ctab": ctab,
                "stab": stab,
                "tri": tri,
                "ident": ident,
                "perm": perm,
            }
        )
    nc = _get_nc()
    res = run_bass_kernel_spmd(
        nc, in_maps, core_ids=list(range(N_CORES)), **run_kw
    )
    out = np.empty((B, S, D), dtype=np.float32)
    for c in range(N_CORES):
        yc = res.results[c]["y"]
        out[0, c * TPB : (c + 1) * TPB, :] = yc[0:TPB]
        out[1, c * TPB : (c + 1) * TPB, :] = yc[TPB:TSL]
    kernel.last_results = res
    return out
